# revision 45
# baseline (speedup 1.0000x reference)
"""Grouped-Query Attention (S=2048, NQ=32, NKV=8, D=128, HID=4096) on 8 TRN2 NeuronCores.

Sharding: tensor-parallel over heads. Core c owns KV head c and its G=4
query heads (rows c*512..(c+1)*512 of Wq, c*128..(c+1)*128 of Wk/Wv, and
columns c*512..(c+1)*512 of Wo).  Each core computes a partial output
(row-parallel Wo); the host sums the 8 partials.

Schedule: the Exp activations (scalar engine, ~690ns per [128,512] tile)
are the binding resource inside an attention block (PE only has 430ns of
score+ctx matmuls per tile), so attention blocks are interleaved with
"filler" matmuls drawn from other stages:

  A: full projections for seq chunks 1,2,3; k/v-only for chunk 0
  B(1) + filler = q-projection of chunk 0
  B(2) + filler = out-projection of chunk 1
  B(3) + filler = out-projection of chunk 2
  B(0) + filler = out-projection of chunk 3
  tail: out-projection of chunk 0

Each attention block slot emits [filler-pair, score(j+3), ctx(j)] so the
PE never head-of-line blocks on the scalar engine.  Softmax row-sums
accumulate in bf16 on DVE (sequential rounding errors average out across
the 128 partitions inside the broadcast ones-matmul), and 1/denominator
uses the fast custom-DVE reciprocal.  Output is written bf16; the host
accumulates the 8 partials in fp32.

HW notes (measured here): fp16 matmul operands give garbage; gpsimd+f32r
fails codegen; fp8 DoubleRow is only ~1.44x so bf16 everywhere is optimal
under the 2e-2 gate; startup is DMA-queue-order-critical (big weight DMAs
must not be queued ahead of the first x tiles).
"""

import os
import sys

import numpy as np
import ml_dtypes

for _p in ("/opt/trn_rl_repo", "/root/.axon_site/_ro/trn_rl_repo"):
    if os.path.isdir(_p) and _p not in sys.path:
        sys.path.insert(0, _p)

import concourse.bass as bass
import concourse.bacc as bacc
import concourse.mybir as mybir
import concourse.tile as tile
from concourse.bass_utils import run_bass_kernel_spmd
from concourse.masks import make_identity

P = 128          # partitions / head dim / PE tile
S = 2048         # sequence length
HID = 4096       # hidden dim
NCORES = 8
NH = 4           # q heads per core
DQ = NH * P      # per-core q width (512)
SC = 512         # free-dim chunk (PSUM bank = 512 fp32)
NKT = HID // P   # 32 contraction tiles over hidden
NCH = S // SC    # 4 sequence chunks
NJT = S // P     # 16 key tiles
NOC = HID // SC  # 8 out column chunks
SCALE = float(P) ** -0.5
BF = mybir.dt.bfloat16
F16 = mybir.dt.float16
F32 = mybir.dt.float32
BFNP = np.dtype(ml_dtypes.bfloat16)
F16NP = np.dtype(ml_dtypes.float16) if hasattr(ml_dtypes, "float16") else np.dtype(np.float16)

_CACHE = {}

_RACC_SPLIT = os.environ.get("K_RACC_SPLIT", "0") == "1"
_RECIP_FAST = os.environ.get("K_RECIP_FAST", "1") == "1"
_RACC_DT_NAME = os.environ.get("K_RACC_DT", "bf16")


class _Filler:
    """Queue of emission steps (~2 matmuls each) interleaved into B blocks."""

    def __init__(self, steps):
        self.steps = list(steps)

    def take(self, n=1):
        for _ in range(n):
            if self.steps:
                self.steps.pop(0)()

    def drain(self):
        while self.steps:
            self.steps.pop(0)()


def _build():
    global _RACC_DT
    _RACC_DT = {"f16": F16, "bf16": BF, "f32r": mybir.dt.float32r}[_RACC_DT_NAME]
    nc = bacc.Bacc(None, target_bir_lowering=False)
    xT = nc.declare_dram_parameter("xT", [HID, S], BF, isOutput=False)
    WqT = nc.declare_dram_parameter("WqT", [HID, DQ], BF, isOutput=False)
    WkT = nc.declare_dram_parameter("WkT", [HID, P], BF, isOutput=False)
    WvT = nc.declare_dram_parameter("WvT", [HID, P], BF, isOutput=False)
    bvp = nc.declare_dram_parameter("bvp", [P, 1], F32, isOutput=False)
    WoT = nc.declare_dram_parameter("WoT", [DQ, HID], BF, isOutput=False)
    _ONES_DT = {"f16": F16, "bf16": BF, "f32r": mybir.dt.float32r}[_RACC_DT_NAME]
    onesd = nc.declare_dram_parameter("onesd", [P, P], _ONES_DT, isOutput=False)
    out = nc.declare_dram_parameter("out", [S, HID], BF, isOutput=True)

    with tile.TileContext(nc) as tc:
        with (
            tc.tile_pool(name="consts", bufs=1) as consts,
            tc.tile_pool(name="acts", bufs=1) as acts,
            tc.tile_pool(name="xin", bufs=12) as xin,
            tc.tile_pool(name="qxin", bufs=6) as qxin,
            tc.tile_pool(name="epool", bufs=6) as epool,
            tc.tile_pool(name="rpool", bufs=2) as rpool,
            tc.tile_pool(name="opool", bufs=10) as opool,
        ):
            # ---- constants: small weights first so stage A starts early ----
            ident = consts.tile([P, P], BF)
            make_identity(nc, ident)
            bv_sb = consts.tile([P, 1], F32)
            nc.sync.dma_start(out=bv_sb, in_=bvp[:, :])
            wk = consts.tile([P, NKT, P], BF)
            nc.sync.dma_start(out=wk, in_=WkT[:, :].rearrange("(kt p) d -> p kt d", p=P))
            wv = consts.tile([P, NKT, P], BF)
            wq = consts.tile([P, NKT, DQ], BF)
            wq_src = WqT[:, :].rearrange("(g kt p) d -> p g kt d", p=P, g=4)
            ones16 = consts.tile([P, P], _ONES_DT, name="ones16")
            # wv/wq/ones/wo are DMA'd interleaved with the first x tiles so
            # the startup-critical bytes (wk + first x) aren't queued behind
            wo = consts.tile([P, NH, HID], BF)

            # ---- persistent activations ----
            qT = acts.tile([P, NH, S], BF)      # per head: [128 d, 2048 s]
            kT = acts.tile([P, S], BF)          # [128 d, 2048 s]
            vT = acts.tile([P, S], BF)          # [128 d, 2048 s]
            v = acts.tile([P, NJT, P], BF)      # [128 j, jt, 128 d]
            ctxT = acts.tile([P, NH, S], BF)    # per head: [128 d, 2048 i]

            # ---- PE warmup: cover initial weight DMA latency, release HAM.
            # The first batch reads an uninitialized tile (no writer => no
            # wait) so the PE starts immediately instead of waiting for
            # make_identity; the results land in an unread PSUM tile. ----
            with tc.tile_pool(name="pwarm", bufs=1, space="PSUM") as pwarm:
                wt = pwarm.tile([P, P], BF, name="warm")
                for _ in range(48):
                    nc.tensor.transpose(wt, ident, ident)

            # ---- stage A ----
            def chunk_x(c, pool, extra=None):
                """DMA the 32 k-tiles of x for seq chunk c as 16 paired tiles."""
                s0 = c * SC
                tiles = []
                for kp in range(NKT // 2):
                    xt = pool.tile([P, 2, SC], BF, name="xt")
                    nc.sync.dma_start(
                        out=xt,
                        in_=xT[2 * kp * P:(2 * kp + 2) * P, s0:s0 + SC].rearrange(
                            "(t p) s -> p t s", p=P))
                    tiles.append(xt)
                    if extra and kp in extra:
                        extra[kp]()
                return tiles

            def evict_kv(c, k_ps, v_ps):
                s0 = c * SC
                nc.scalar.copy(out=kT[:, s0:s0 + SC], in_=k_ps)
                nc.scalar.activation(out=vT[:, s0:s0 + SC], in_=v_ps,
                                     func=mybir.ActivationFunctionType.Identity,
                                     bias=bv_sb, scale=1.0)

            def transpose_v(c, pool):
                for jj in range(SC // P):
                    jt = c * (SC // P) + jj
                    t_ps = pool.tile([P, P], BF, tag="ptr", bufs=2, name="t_ps")
                    nc.tensor.transpose(t_ps, vT[:, jt * P:(jt + 1) * P], ident)
                    nc.vector.tensor_copy(out=v[:, jt, :], in_=t_ps)

            with tc.tile_pool(name="pa", bufs=1, space="PSUM") as pa:
                # full projections for chunks 1..3.  Chunk 1 (the first one
                # emitted) lags its q matmuls by QLAG k-tiles so the k/v
                # matmuls can start after only wk + the first x pair arrive.
                def _wq_piece(g):
                    return lambda: nc.sync.dma_start(
                        out=wq[:, g * 8:(g + 1) * 8, :], in_=wq_src[:, g])
                for c in (1, 2, 3):
                    s0 = c * SC
                    q_ps = [pa.tile([P, SC], F32, tag="pq%d" % m, name="q_ps%d" % m)
                            for m in range(NH)]
                    k_ps = pa.tile([P, SC], F32, tag="pk")
                    v_ps = pa.tile([P, SC], F32, tag="pv")
                    if c == 1:
                        lag = 8
                        # wv after the first x pair, wq_g0 soon after, rest spread
                        extra = {
                            0: lambda: nc.sync.dma_start(
                                out=wv,
                                in_=WvT[:, :].rearrange("(kt p) d -> p kt d", p=P)),
                            1: _wq_piece(0),
                            2: lambda: nc.sync.dma_start(out=ones16, in_=onesd[:, :]),
                            4: _wq_piece(1),
                            7: _wq_piece(2),
                            10: _wq_piece(3),
                        }
                        xts = chunk_x(c, xin, extra=extra)
                    elif c == 2:
                        lag = 0
                        xts = chunk_x(c, xin, extra={0: lambda: nc.sync.dma_start(
                            out=wo,
                            in_=WoT[:, :].rearrange("(dt p) o -> p dt o", p=P))})
                    else:
                        lag = 0
                        xts = chunk_x(c, xin)
                    warm2 = None
                    if lag:
                        warm2 = pa.tile([P, P], BF, tag="ptr", bufs=2, name="warm2")
                    for kt in range(NKT + lag):
                        if kt < NKT:
                            xt = xts[kt // 2][:, kt % 2, :]
                            st, sp = kt == 0, kt == NKT - 1
                            nc.tensor.matmul(k_ps, lhsT=wk[:, kt, :], rhs=xt,
                                             start=st, stop=sp)
                            nc.tensor.matmul(v_ps, lhsT=wv[:, kt, :], rhs=xt,
                                             start=st, stop=sp)
                        if lag and kt < lag:
                            # keep the PE warm (and the x-consumption rate at
                            # DMA pace) during the q-less prefix of chunk 1
                            nc.tensor.transpose(warm2, ident, ident)
                            nc.tensor.transpose(warm2, ident, ident)
                        kq = kt - lag
                        if kq >= 0:
                            xq = xts[kq // 2][:, kq % 2, :]
                            st, sp = kq == 0, kq == NKT - 1
                            for m in range(NH):
                                nc.tensor.matmul(q_ps[m],
                                                 lhsT=wq[:, kq, m * P:(m + 1) * P],
                                                 rhs=xq, start=st, stop=sp)
                    evict_kv(c, k_ps, v_ps)
                    for m in range(NH):
                        nc.vector.tensor_copy(out=qT[:, m, s0:s0 + SC], in_=q_ps[m])
                    transpose_v(c, pa)
                # k/v-only for chunk 0 (its q runs as B(1) filler)
                k_ps = pa.tile([P, SC], F32, tag="pk", name="k_ps0")
                v_ps = pa.tile([P, SC], F32, tag="pv", name="v_ps0")
                xts = chunk_x(0, xin)
                for kt in range(NKT):
                    xt = xts[kt // 2][:, kt % 2, :]
                    st, sp = kt == 0, kt == NKT - 1
                    nc.tensor.matmul(k_ps, lhsT=wk[:, kt, :], rhs=xt, start=st, stop=sp)
                    nc.tensor.matmul(v_ps, lhsT=wv[:, kt, :], rhs=xt, start=st, stop=sp)
                evict_kv(0, k_ps, v_ps)
                transpose_v(0, pa)

            # ---- stages B + C + q-chunk-0, interleaved ----
            with tc.tile_pool(name="pbc", bufs=1, space="PSUM") as pbc:

                def q0_filler_steps():
                    """q-projection of chunk 0 as 64 steps (2 half-passes of
                    2 heads x 32 k-tiles); one step = 2 matmuls."""
                    steps = []
                    state = {}

                    def step(half, kt):
                        def go():
                            h0, h1 = 2 * half, 2 * half + 1
                            if kt == 0:
                                state["pA"] = pbc.tile([P, SC], F32, tag="pfA",
                                                       bufs=2, name="qf_a")
                                state["pB"] = pbc.tile([P, SC], F32, tag="pfB",
                                                       name="qf_b")
                            if kt % 2 == 0:
                                xt = qxin.tile([P, 2, SC], BF, name="qxt")
                                nc.sync.dma_start(
                                    out=xt,
                                    in_=xT[kt * P:(kt + 2) * P, 0:SC].rearrange(
                                        "(t p) s -> p t s", p=P))
                                state["xt"] = xt
                            xt = state["xt"][:, kt % 2, :]
                            st, sp = kt == 0, kt == NKT - 1
                            nc.tensor.matmul(state["pA"], lhsT=wq[:, kt, h0 * P:(h0 + 1) * P],
                                             rhs=xt, start=st, stop=sp)
                            nc.tensor.matmul(state["pB"], lhsT=wq[:, kt, h1 * P:(h1 + 1) * P],
                                             rhs=xt, start=st, stop=sp)
                            if kt == NKT - 1:
                                nc.vector.tensor_copy(out=qT[:, h0, 0:SC], in_=state["pA"])
                                nc.vector.tensor_copy(out=qT[:, h1, 0:SC], in_=state["pB"])
                        return go

                    for half in range(2):
                        for kt in range(NKT):
                            steps.append(step(half, kt))
                    return steps

                def c_chunk_steps(t, evict_split=False):
                    """out-projection rows of seq chunk t as 64 steps of 2
                    accumulating matmuls.  evict_split alternates evictions
                    between DVE and ACT (used in the tail where ACT is idle)."""
                    steps = []
                    state = {}

                    def step(mt, oc, phase):
                        def go():
                            m0, o0 = mt * P, oc * SC
                            if phase == 0:
                                state["o_ps"] = pbc.tile([P, SC], F32, tag="pfA",
                                                         bufs=2, name="o_ps")
                                for dt_ in (0, 1):
                                    nc.tensor.matmul(state["o_ps"],
                                                     lhsT=ctxT[:, dt_, m0:m0 + P],
                                                     rhs=wo[:, dt_, o0:o0 + SC],
                                                     start=dt_ == 0, stop=False)
                            else:
                                for dt_ in (2, 3):
                                    nc.tensor.matmul(state["o_ps"],
                                                     lhsT=ctxT[:, dt_, m0:m0 + P],
                                                     rhs=wo[:, dt_, o0:o0 + SC],
                                                     start=False, stop=dt_ == 3)
                                ob = opool.tile([P, SC], BF, name="ob")
                                if evict_split and oc % 2 == 1:
                                    nc.scalar.copy(out=ob, in_=state["o_ps"])
                                else:
                                    nc.vector.tensor_copy(out=ob, in_=state["o_ps"])
                                nc.sync.dma_start(
                                    out=out[m0:m0 + P, o0:o0 + SC], in_=ob)
                        return go

                    for mt in range(t * NCH, (t + 1) * NCH):
                        for oc in range(NOC):
                            steps.append(step(mt, oc, 0))
                            steps.append(step(mt, oc, 1))
                    return steps

                def attn_block(t, filler, jorder=None):
                    i0 = t * SC
                    jseq = jorder if jorder is not None else list(range(NJT))
                    for h in range(NH):
                        ctx_ps = pbc.tile([P, SC], F32, tag="pctx", bufs=2,
                                          name="ctx_ps")
                        racc_a = rpool.tile([P, SC], _RACC_DT, name="racc_a", bufs=2)
                        racc_b = None
                        if _RACC_SPLIT:
                            racc_b = rpool.tile([P, SC], _RACC_DT, name="racc_b",
                                                bufs=2)
                            nc.gpsimd.memset(racc_b, 0.0)
                        e_tiles = [None] * NJT

                        def emit_score(j):
                            jt = jseq[j]
                            s_ps = pbc.tile([P, SC], F32, tag="pscore", bufs=3,
                                            name="s_ps")
                            nc.tensor.matmul(s_ps, lhsT=kT[:, jt * P:(jt + 1) * P],
                                             rhs=qT[:, h, i0:i0 + SC],
                                             start=True, stop=True)
                            e_sb = epool.tile([P, SC], BF, name="e_sb")
                            nc.scalar.activation(out=e_sb, in_=s_ps,
                                                 func=mybir.ActivationFunctionType.Exp,
                                                 scale=SCALE)
                            e_tiles[j] = e_sb
                            # rowsum accumulation, split DVE / Pool
                            if _RACC_SPLIT:
                                if j % 2 == 0:
                                    if j == 0:
                                        nc.vector.tensor_copy(out=racc_a, in_=e_sb)
                                    else:
                                        nc.vector.tensor_add(out=racc_a, in0=racc_a,
                                                             in1=e_sb)
                                else:
                                    nc.gpsimd.tensor_add(out=racc_b, in0=racc_b,
                                                         in1=e_sb)
                            else:
                                if j == 0:
                                    nc.vector.tensor_copy(out=racc_a, in_=e_sb)
                                else:
                                    nc.vector.tensor_add(out=racc_a, in0=racc_a,
                                                         in1=e_sb)

                        for j in range(3):
                            emit_score(j)
                        for j in range(NJT):
                            filler.take(1)
                            if j + 3 < NJT:
                                emit_score(j + 3)
                            nc.tensor.matmul(ctx_ps, lhsT=v[:, jseq[j], :],
                                             rhs=e_tiles[j],
                                             start=j == 0, stop=j == NJT - 1)
                        # denominator: broadcast column sums of both racc halves
                        rb_ps = pbc.tile([P, SC], F32, tag="pscore", bufs=3,
                                         name="rb_ps")
                        if _RACC_SPLIT:
                            nc.tensor.matmul(rb_ps, lhsT=ones16, rhs=racc_a,
                                             start=True, stop=False)
                            nc.tensor.matmul(rb_ps, lhsT=ones16, rhs=racc_b,
                                             start=False, stop=True)
                        else:
                            nc.tensor.matmul(rb_ps, lhsT=ones16, rhs=racc_a,
                                             start=True, stop=True)
                        rbc = rpool.tile([P, SC], F32, name="rbc", bufs=2)
                        if _RECIP_FAST:
                            nc.vector.reciprocal_approx_fast(out=rbc, in_=rb_ps)
                        else:
                            nc.vector.reciprocal(out=rbc, in_=rb_ps)
                        nc.vector.tensor_mul(out=ctxT[:, h, i0:i0 + SC],
                                             in0=ctx_ps, in1=rbc)

                # B(1) starts with chunk-1/2/3 keys so it doesn't wait on the
                # (last-evicted) chunk-0 kT/v at the stage-A boundary
                fill1 = _Filler(q0_filler_steps())
                attn_block(1, fill1, jorder=list(range(4, NJT)) + list(range(4)))
                fill1.drain()
                for t, csrc in ((2, 1), (3, 2), (0, 3)):
                    f = _Filler(c_chunk_steps(csrc))
                    attn_block(t, f)
                    f.drain()
                # tail: out-projection of chunk 0
                f = _Filler(c_chunk_steps(0, evict_split=True))
                f.drain()
    nc.finalize()
    return nc


def _get_program():
    if "nc" not in _CACHE:
        _CACHE["nc"] = _build()
    return _CACHE["nc"]


def _prep_inputs(hidden_states, Wq, Wk, Wv, bv, Wo):
    x = np.asarray(hidden_states, np.float32).reshape(S, HID)
    xT = np.ascontiguousarray(x.T).astype(BFNP)
    Wq = np.asarray(Wq, np.float32)
    Wk = np.asarray(Wk, np.float32)
    Wv = np.asarray(Wv, np.float32)
    bv = np.asarray(bv, np.float32)
    Wo = np.asarray(Wo, np.float32)
    maps = []
    for c in range(NCORES):
        qs = slice(c * DQ, (c + 1) * DQ)
        ks = slice(c * P, (c + 1) * P)
        maps.append({
            "xT": xT,
            "WqT": np.ascontiguousarray(Wq[qs].T).astype(BFNP),
            "WkT": np.ascontiguousarray(Wk[ks].T).astype(BFNP),
            "WvT": np.ascontiguousarray(Wv[ks].T).astype(BFNP),
            "bvp": np.ascontiguousarray(bv[ks]).reshape(P, 1),
            "WoT": np.ascontiguousarray(Wo[:, qs].T).astype(BFNP),
            "onesd": np.ones((P, P),
                             {"f16": F16NP, "bf16": BFNP,
                              "f32r": np.dtype(np.float32)}[_RACC_DT_NAME]),
        })
    return maps


def kernel(hidden_states, Wq, Wk, Wv, bv, Wo, _trace=False, **kw):
    nc = _get_program()
    maps = _prep_inputs(hidden_states, Wq, Wk, Wv, bv, Wo)
    res = run_bass_kernel_spmd(nc, maps, list(range(NCORES)), trace=_trace, **kw)
    out = np.zeros((S, HID), np.float32)
    for c in range(NCORES):
        out += np.asarray(res.results[c]["out"], np.float32)
    if _trace:
        return out.reshape(1, S, HID), res
    return out.reshape(1, S, HID)
